# revision 8
# baseline (speedup 1.0000x reference)
"""Trainium2 Bass kernel for nn_AdaptivePoolingClassifier (8 NeuronCores).

Math: the reference MLP is linear up to its single ReLU, so W1..W3 fold
into one 128x128 matrix on the host:
    h   = relu(x @ Wc^T + bc)       Wc = W3 W2 W1 ; bc = W3(W2 b1+b2)+b3
    p   = h @ W4^T + b4
    out = sum_n p * softmax(alpha*p, axis=n)

Device computes (rows sharded 8 ways):
    pt  = h @ (diag(alpha) W4)^T        # = alpha*(p - b4), [rows, 5]
    den_partial = sum_rows exp(pt) ; num_partial = sum_rows pt*exp(pt)
Host finishes: out_o = num_o/(alpha_o*den_o) + b4_o (the softmax is
invariant to the per-column constant factor exp(-alpha_o*b4_o)).

v2 layout/schedule:
  - x is transposed on the host to [128(feat), rows] and quantized to
    fp8e4 (halves HBM traffic; weights stay bf16 -- mixed-dtype matmul).
    Row-quantization noise averages out over the 200k-row softmax pool,
    measured rel err ~1.4e-3 (better than all-bf16).
  - L1: wct stationary, stream x in 512-col psum banks, TILE=1536.
  - ReLU split ACT/DVE by chunk-aligned column ranges (7/5 of 12).
  - L4: h chunks (128 rows) as matmul stationary so pt lands
    rows-on-partitions; runs 2 tiles behind L1 so the stationary is
    long-ready.  Pooling in batches of 96 chunks: ACT exp, DVE mult +
    num-reduce, GpSimd den-reduce, per-batch DMA of partials.
"""

import numpy as np
import ml_dtypes

from concourse import bacc, mybir, tile
from concourse.bass_utils import run_bass_kernel_spmd

N_CORES = 8
N_ROWS = 200000
F = 128
OUT = 5

ROWS_PAD = 200704            # 8 * 25088
RPC = ROWS_PAD // N_CORES    # rows per core = 25088
T0 = 512                     # prologue tile
TILE = 1536                  # steady-state compute tile (3 psum banks)
N_TILES = (RPC - T0) // TILE  # 16
GROUP = 3072                 # rows per steady-state DMA (2 tiles)
N_GROUPS = (RPC - T0) // GROUP  # 8
CHUNK = 128                  # rows per layer-4 matmul (stationary M)
N_CHUNKS = RPC // CHUNK      # 196
SLOTS = 96                   # pt chunks per pooling batch
N_BATCH = (N_CHUNKS + SLOTS - 1) // SLOTS  # 3 (96+96+4)
ACT_CH = 7                   # relu chunks on ACT per 1536 tile (rest DVE)

F32 = mybir.dt.float32
BF16 = mybir.dt.bfloat16
FP8 = mybir.dt.float8e4
AF = mybir.ActivationFunctionType
ALU = mybir.AluOpType


def build_bass(has_bias=False):
    nc = bacc.Bacc()

    CONST_COLS = (F + OUT + 1) if has_bias else (F + OUT)
    cst_ext = nc.declare_dram_parameter(
        "cst", [F, CONST_COLS], BF16, isOutput=False
    )
    xt_ext = nc.declare_dram_parameter("xt", [F, RPC], FP8, isOutput=False)
    out_ext = nc.declare_dram_parameter(
        "out", [F, N_BATCH, 2, OUT], F32, isOutput=True
    )

    with tile.TileContext(nc) as tc:
        with (
            tc.tile_pool(name="scratch", bufs=1) as scratch,
            tc.tile_pool(name="accs", bufs=1) as accs,
            tc.tile_pool(name="x0p", bufs=1) as x0p,
            tc.tile_pool(name="xin", bufs=3) as xin,
            tc.tile_pool(name="hbufl", bufs=3) as hbufl,
            tc.tile_pool(name="hbufr", bufs=3) as hbufr,
            tc.tile_pool(name="ebuf", bufs=2) as ebuf,
            tc.tile_pool(name="ps_h", bufs=2, space="PSUM") as ps_h,
            tc.tile_pool(name="ps_p", bufs=2, space="PSUM") as ps_p,
        ):
            parts = accs.tile([F, N_BATCH, 2, OUT], F32)

            cstt = scratch.tile([F, CONST_COLS], BF16)
            nc.sync.dma_start(out=cstt[:], in_=cst_ext[:])
            wct = cstt[:, :F]
            w4at = cstt[:, F : F + OUT]
            nc.tensor.ldweights(wct)  # PE observes the const DMA early
            bc = None
            if has_bias:
                bc = scratch.tile([F, 1], F32)
                nc.vector.tensor_copy(bc[:], cstt[:, F + OUT : F + OUT + 1])

            x0 = x0p.tile([F, T0], FP8)
            nc.sync.dma_start(out=x0[:], in_=xt_ext[:, :T0])
            xg = []
            for g in range(N_GROUPS):
                t = xin.tile([F, GROUP], FP8)
                nc.sync.dma_start(
                    out=t[:], in_=xt_ext[:, T0 + g * GROUP : T0 + (g + 1) * GROUP]
                )
                xg.append(t)

            state = {"chunk": 0, "pp": None}
            hbufs = []  # per tile: (hl, hr, a_ch, n_ch)

            def flush_batch(n_slots):
                bi = (state["chunk"] - 1) // SLOTS
                pp = state["pp"]
                sl = slice(0, n_slots)
                e_b = ebuf.tile([F, OUT, SLOTS], BF16, tag="e_b")
                pe_b = ebuf.tile([F, OUT, SLOTS], BF16, tag="pe_b")
                # per-o exp so ACT's accum_out yields the den partial sums
                for o in range(OUT):
                    nc.scalar.activation(
                        e_b[:, o, sl], pp[:, o, sl], AF.Exp,
                        accum_out=parts[:, bi, 0, o : o + 1],
                    )
                nc.vector.tensor_tensor(
                    pe_b[:, :, sl], pp[:, :, sl], e_b[:, :, sl], ALU.mult
                )
                nc.vector.tensor_reduce(
                    parts[:, bi, 1, :], pe_b[:, :, sl],
                    mybir.AxisListType.X, ALU.add,
                )
                nc.sync.dma_start(
                    out=out_ext[:, bi], in_=parts[:, bi]
                )

            def do_l1(rhs, rows):
                n_ch = rows // CHUNK
                a_ch = min(ACT_CH, n_ch // 2 + 1) if rows < TILE else ACT_CH
                a_cols = a_ch * CHUNK
                h3p = ps_h.tile([F, TILE], F32, tag="h3p", name="h3p")
                for c0 in range(0, rows, 512):
                    cw = min(512, rows - c0)
                    nc.tensor.matmul(
                        h3p[:, c0 : c0 + cw], wct, rhs[:, c0 : c0 + cw],
                        start=True, stop=True,
                    )
                hl = hbufl.tile([F, ACT_CH * CHUNK], BF16, tag="hl")
                hr = hbufr.tile([F, (TILE // CHUNK - ACT_CH) * CHUNK], BF16, tag="hr")
                if a_ch:
                    if has_bias:
                        nc.scalar.activation(
                            hl[:, :a_cols], h3p[:, :a_cols], AF.Relu,
                            bias=bc[:], scale=1.0,
                        )
                    else:
                        nc.scalar.activation(
                            hl[:, :a_cols], h3p[:, :a_cols], AF.Relu
                        )
                if a_cols < rows:
                    if has_bias:
                        nc.vector.tensor_scalar(
                            hr[:, : rows - a_cols], h3p[:, a_cols:rows],
                            bc[:], 0.0, ALU.add, ALU.max,
                        )
                    else:
                        nc.vector.tensor_scalar_max(
                            hr[:, : rows - a_cols], h3p[:, a_cols:rows], 0.0
                        )
                hbufs.append((hl, hr, a_ch, n_ch))

            def do_l4(ti):
                hl, hr, a_ch, n_ch = hbufs[ti]
                for j in range(n_ch):
                    c = state["chunk"]
                    s = c % SLOTS
                    if s == 0:
                        state["pp"] = ps_p.tile(
                            [F, OUT, SLOTS], F32, tag="pp", name="pp"
                        )
                    if j < a_ch:
                        lhs = hl[:, j * CHUNK : (j + 1) * CHUNK]
                    else:
                        lhs = hr[:, (j - a_ch) * CHUNK : (j - a_ch + 1) * CHUNK]
                    nc.tensor.matmul(
                        state["pp"][:, :, s], lhs, w4at,
                        start=True, stop=True,
                    )
                    state["chunk"] = c + 1
                    if s == SLOTS - 1 or state["chunk"] == N_CHUNKS:
                        flush_batch(s + 1)

            do_l1(x0[:], T0)
            for t in range(N_TILES):
                g, off = divmod(t * TILE, GROUP)
                do_l1(xg[g][:, off : off + TILE], TILE)
                if t >= 1:
                    do_l4(t - 1)  # tile index in hbufs: t0 is 0
            do_l4(N_TILES - 1)
            do_l4(N_TILES)

    nc.finalize()
    return nc


_CACHED = {}
TRACE = False
LAST = {}


def kernel(x, W1, b1, W2, b2, W3, b3, W4, b4, alpha):
    f64 = np.float64
    x2 = np.asarray(x, np.float32).reshape(N_ROWS, F)
    W1, b1, W2, b2, W3, b3, W4, b4, alpha = [
        np.asarray(a, f64) for a in (W1, b1, W2, b2, W3, b3, W4, b4, alpha)
    ]

    # fold the linear layers (exact in f64)
    Wc = W3 @ W2 @ W1
    bc = W3 @ (W2 @ b1 + b2) + b3
    alpha_safe = np.where(np.abs(alpha) < 1e-12, 1e-12, alpha)
    W4a = alpha_safe[:, None] * W4

    # pad rows to 8*25088 with zeros; contribution removed on the host
    n_pad = ROWS_PAD - N_ROWS
    xp = np.concatenate([x2, np.zeros((n_pad, F), np.float32)], axis=0)
    xT = np.ascontiguousarray(xp.T).astype(ml_dtypes.float8_e4m3fn)

    has_bias = bool(np.any(bc != 0.0))
    key = ("nc", has_bias)
    if key not in _CACHED:
        _CACHED[key] = build_bass(has_bias)
    nc = _CACHED[key]

    wct_np = np.ascontiguousarray(Wc.T).astype(ml_dtypes.bfloat16)
    w4at_np = np.ascontiguousarray(W4a.T).astype(ml_dtypes.bfloat16)
    parts_list = [wct_np, w4at_np]
    if has_bias:
        parts_list.append(
            bc.reshape(F, 1).astype(np.float32).astype(ml_dtypes.bfloat16)
        )
    consts_np = np.concatenate(parts_list, axis=1)

    in_maps = []
    for c in range(N_CORES):
        shard = np.ascontiguousarray(xT[:, c * RPC : (c + 1) * RPC])
        in_maps.append({"cst": consts_np, "xt": shard})

    res = run_bass_kernel_spmd(
        nc, in_maps, core_ids=list(range(N_CORES)), trace=TRACE
    )
    LAST["res"] = res
    outs = np.stack(
        [np.asarray(r["out"], f64) for r in res.results]
    )  # [8, F, NB, 2, OUT]
    den = outs[:, :, :, 0, :].sum(axis=(0, 1, 2))  # [5]
    num = outs[:, :, :, 1, :].sum(axis=(0, 1, 2))  # [5]

    # remove the zero-pad rows' contribution (each pad row: h0 = relu(bc))
    h0 = np.maximum(bc, 0.0)
    pt0 = W4a @ h0
    den -= n_pad * np.exp(pt0)
    num -= n_pad * pt0 * np.exp(pt0)

    out = num / (alpha_safe * den) + b4
    return out[None, :].astype(np.float32)


# revision 9
# speedup vs baseline: 1.0347x; 1.0347x over previous
"""Trainium2 Bass kernel for nn_AdaptivePoolingClassifier (8 NeuronCores).

Math: the reference MLP is linear up to its single ReLU, so W1..W3 fold
into one 128x128 matrix on the host:
    h   = relu(x @ Wc^T + bc)       Wc = W3 W2 W1 ; bc = W3(W2 b1+b2)+b3
    p   = h @ W4^T + b4
    out = sum_n p * softmax(alpha*p, axis=n)

Device computes (rows sharded 8 ways):
    pt  = h @ (diag(alpha) W4)^T        # = alpha*(p - b4), [rows, 5]
    den_partial = sum_rows exp(pt) ; num_partial = sum_rows pt*exp(pt)
Host finishes: out_o = num_o/(alpha_o*den_o) + b4_o (the softmax is
invariant to the per-column constant factor exp(-alpha_o*b4_o)).

v2 layout/schedule:
  - x is transposed on the host to [128(feat), rows] and quantized to
    fp8e4 (halves HBM traffic; weights stay bf16 -- mixed-dtype matmul).
    Row-quantization noise averages out over the 200k-row softmax pool,
    measured rel err ~1.4e-3 (better than all-bf16).
  - L1: wct stationary, stream x in 512-col psum banks, TILE=1536.
  - ReLU split ACT/DVE by chunk-aligned column ranges (7/5 of 12).
  - L4: h chunks (128 rows) as matmul stationary so pt lands
    rows-on-partitions; runs 2 tiles behind L1 so the stationary is
    long-ready.  Pooling in batches of 96 chunks: ACT exp, DVE mult +
    num-reduce, GpSimd den-reduce, per-batch DMA of partials.
"""

import numpy as np
import ml_dtypes

from concourse import bacc, mybir, tile
from concourse.bass_utils import run_bass_kernel_spmd

N_CORES = 8
N_ROWS = 200000
F = 128
OUT = 5

ROWS_PAD = 200704            # 8 * 25088
RPC = ROWS_PAD // N_CORES    # rows per core = 25088
T0 = 512                     # prologue tile
TILE = 1536                  # steady-state compute tile (3 psum banks)
N_TILES = (RPC - T0) // TILE  # 16
GROUP = 3072                 # rows per steady-state DMA (2 tiles)
N_GROUPS = (RPC - T0) // GROUP  # 8
CHUNK = 128                  # rows per layer-4 matmul (stationary M)
N_CHUNKS = RPC // CHUNK      # 196
SLOTS = 96                   # pt chunks per pooling batch
N_BATCH = (N_CHUNKS + SLOTS - 1) // SLOTS  # 3 (96+96+4)
ACT_CH = 7                   # relu chunks on ACT per 1536 tile (rest DVE)

F32 = mybir.dt.float32
BF16 = mybir.dt.bfloat16
FP8 = mybir.dt.float8e4
AF = mybir.ActivationFunctionType
ALU = mybir.AluOpType


def build_bass(has_bias=False):
    nc = bacc.Bacc()

    CONST_COLS = (F + OUT + 1) if has_bias else (F + OUT)
    cst_ext = nc.declare_dram_parameter(
        "cst", [F, CONST_COLS], BF16, isOutput=False
    )
    xt_ext = nc.declare_dram_parameter("xt", [F, RPC], FP8, isOutput=False)
    out_ext = nc.declare_dram_parameter(
        "out", [F, N_BATCH, 2, OUT], F32, isOutput=True
    )

    with tile.TileContext(nc) as tc:
        with (
            tc.tile_pool(name="scratch", bufs=1) as scratch,
            tc.tile_pool(name="accs", bufs=1) as accs,
            tc.tile_pool(name="x0p", bufs=1) as x0p,
            tc.tile_pool(name="xin", bufs=3) as xin,
            tc.tile_pool(name="hbufl", bufs=3) as hbufl,
            tc.tile_pool(name="hbufr", bufs=3) as hbufr,
            tc.tile_pool(name="ebuf", bufs=2) as ebuf,
            tc.tile_pool(name="ps_h", bufs=2, space="PSUM") as ps_h,
            tc.tile_pool(name="ps_p", bufs=2, space="PSUM") as ps_p,
        ):
            parts = accs.tile([F, N_BATCH, 2, OUT], F32)

            cstt = scratch.tile([F, CONST_COLS], BF16)
            nc.sync.dma_start(out=cstt[:], in_=cst_ext[:])
            wct = cstt[:, :F]
            w4at = cstt[:, F : F + OUT]
            nc.tensor.ldweights(wct)  # PE observes the const DMA early
            bc = None
            if has_bias:
                bc = scratch.tile([F, 1], F32)
                nc.vector.tensor_copy(bc[:], cstt[:, F + OUT : F + OUT + 1])

            x0 = x0p.tile([F, T0], FP8)
            nc.sync.dma_start(out=x0[:], in_=xt_ext[:, :T0])
            xg = []
            for g in range(N_GROUPS):
                t = xin.tile([F, GROUP], FP8)
                nc.sync.dma_start(
                    out=t[:], in_=xt_ext[:, T0 + g * GROUP : T0 + (g + 1) * GROUP]
                )
                xg.append(t)

            state = {"chunk": 0, "pp": None}
            hbufs = []  # per tile: (hl, hr, a_ch, n_ch)

            def flush_batch(n_slots):
                bi = (state["chunk"] - 1) // SLOTS
                pp = state["pp"]
                sl = slice(0, n_slots)
                e_b = ebuf.tile([F, OUT, SLOTS], BF16, tag="e_b")
                pe_b = ebuf.tile([F, OUT, SLOTS], BF16, tag="pe_b")
                nc.scalar.activation(e_b[:, :, sl], pp[:, :, sl], AF.Exp)
                nc.vector.tensor_tensor(
                    pe_b[:, :, sl], pp[:, :, sl], e_b[:, :, sl], ALU.mult
                )
                nc.vector.tensor_reduce(
                    parts[:, bi, 0, :], e_b[:, :, sl],
                    mybir.AxisListType.X, ALU.add,
                )
                nc.vector.tensor_reduce(
                    parts[:, bi, 1, :], pe_b[:, :, sl],
                    mybir.AxisListType.X, ALU.add,
                )
                nc.sync.dma_start(
                    out=out_ext[:, bi], in_=parts[:, bi]
                )

            def do_l1(rhs, rows):
                n_ch = rows // CHUNK
                a_ch = min(ACT_CH, n_ch // 2 + 1) if rows < TILE else ACT_CH
                a_cols = a_ch * CHUNK
                h3p = ps_h.tile([F, TILE], F32, tag="h3p", name="h3p")
                for c0 in range(0, rows, 512):
                    cw = min(512, rows - c0)
                    nc.tensor.matmul(
                        h3p[:, c0 : c0 + cw], wct, rhs[:, c0 : c0 + cw],
                        start=True, stop=True,
                    )
                hl = hbufl.tile([F, ACT_CH * CHUNK], BF16, tag="hl")
                hr = hbufr.tile([F, (TILE // CHUNK - ACT_CH) * CHUNK], BF16, tag="hr")
                if a_ch:
                    if has_bias:
                        nc.scalar.activation(
                            hl[:, :a_cols], h3p[:, :a_cols], AF.Relu,
                            bias=bc[:], scale=1.0,
                        )
                    else:
                        nc.scalar.activation(
                            hl[:, :a_cols], h3p[:, :a_cols], AF.Relu
                        )
                if a_cols < rows:
                    if has_bias:
                        nc.vector.tensor_scalar(
                            hr[:, : rows - a_cols], h3p[:, a_cols:rows],
                            bc[:], 0.0, ALU.add, ALU.max,
                        )
                    else:
                        nc.vector.tensor_scalar_max(
                            hr[:, : rows - a_cols], h3p[:, a_cols:rows], 0.0
                        )
                hbufs.append((hl, hr, a_ch, n_ch))

            def do_l4(ti):
                hl, hr, a_ch, n_ch = hbufs[ti]
                for j in range(n_ch):
                    c = state["chunk"]
                    s = c % SLOTS
                    if s == 0:
                        state["pp"] = ps_p.tile(
                            [F, OUT, SLOTS], F32, tag="pp", name="pp"
                        )
                    if j < a_ch:
                        lhs = hl[:, j * CHUNK : (j + 1) * CHUNK]
                    else:
                        lhs = hr[:, (j - a_ch) * CHUNK : (j - a_ch + 1) * CHUNK]
                    nc.tensor.matmul(
                        state["pp"][:, :, s], lhs, w4at,
                        start=True, stop=True,
                    )
                    state["chunk"] = c + 1
                    if s == SLOTS - 1 or state["chunk"] == N_CHUNKS:
                        flush_batch(s + 1)

            do_l1(x0[:], T0)
            for t in range(N_TILES):
                g, off = divmod(t * TILE, GROUP)
                do_l1(xg[g][:, off : off + TILE], TILE)
                if t >= 1:
                    do_l4(t - 1)  # tile index in hbufs: t0 is 0
            do_l4(N_TILES - 1)
            do_l4(N_TILES)

    nc.finalize()
    return nc


_CACHED = {}
TRACE = False
LAST = {}


def kernel(x, W1, b1, W2, b2, W3, b3, W4, b4, alpha):
    f64 = np.float64
    x2 = np.asarray(x, np.float32).reshape(N_ROWS, F)
    W1, b1, W2, b2, W3, b3, W4, b4, alpha = [
        np.asarray(a, f64) for a in (W1, b1, W2, b2, W3, b3, W4, b4, alpha)
    ]

    # fold the linear layers (exact in f64)
    Wc = W3 @ W2 @ W1
    bc = W3 @ (W2 @ b1 + b2) + b3
    alpha_safe = np.where(np.abs(alpha) < 1e-12, 1e-12, alpha)
    W4a = alpha_safe[:, None] * W4

    # pad rows to 8*25088 with zeros; contribution removed on the host
    n_pad = ROWS_PAD - N_ROWS
    xp = np.concatenate([x2, np.zeros((n_pad, F), np.float32)], axis=0)
    xT = np.ascontiguousarray(xp.T).astype(ml_dtypes.float8_e4m3fn)

    has_bias = bool(np.any(bc != 0.0))
    key = ("nc", has_bias)
    if key not in _CACHED:
        _CACHED[key] = build_bass(has_bias)
    nc = _CACHED[key]

    wct_np = np.ascontiguousarray(Wc.T).astype(ml_dtypes.bfloat16)
    w4at_np = np.ascontiguousarray(W4a.T).astype(ml_dtypes.bfloat16)
    parts_list = [wct_np, w4at_np]
    if has_bias:
        parts_list.append(
            bc.reshape(F, 1).astype(np.float32).astype(ml_dtypes.bfloat16)
        )
    consts_np = np.concatenate(parts_list, axis=1)

    in_maps = []
    for c in range(N_CORES):
        shard = np.ascontiguousarray(xT[:, c * RPC : (c + 1) * RPC])
        in_maps.append({"cst": consts_np, "xt": shard})

    res = run_bass_kernel_spmd(
        nc, in_maps, core_ids=list(range(N_CORES)), trace=TRACE
    )
    LAST["res"] = res
    outs = np.stack(
        [np.asarray(r["out"], f64) for r in res.results]
    )  # [8, F, NB, 2, OUT]
    den = outs[:, :, :, 0, :].sum(axis=(0, 1, 2))  # [5]
    num = outs[:, :, :, 1, :].sum(axis=(0, 1, 2))  # [5]

    # remove the zero-pad rows' contribution (each pad row: h0 = relu(bc))
    h0 = np.maximum(bc, 0.0)
    pt0 = W4a @ h0
    den -= n_pad * np.exp(pt0)
    num -= n_pad * pt0 * np.exp(pt0)

    out = num / (alpha_safe * den) + b4
    return out[None, :].astype(np.float32)


# revision 12
# speedup vs baseline: 1.2531x; 1.2112x over previous
"""Trainium2 Bass kernel for nn_AdaptivePoolingClassifier (8 NeuronCores).

Math: the reference MLP is linear up to its single ReLU, so W1..W3 fold
into one 128x128 matrix on the host:
    h   = relu(x @ Wc^T + bc)       Wc = W3 W2 W1 ; bc = W3(W2 b1+b2)+b3
    p   = h @ W4^T + b4
    out = sum_n p * softmax(alpha*p, axis=1)

Device computes pt = h @ (diag(alpha) W4)^T = alpha*(p - b4) for every
row (rows sharded 8 ways) and streams pt back to DRAM; the host finishes
the softmax pooling in f64 (num/den sums over rows) exactly as it
already finishes the fold / bias algebra.  The softmax weights are
invariant to the per-column constant alpha*b4 shift.

Device layout/schedule (v4):
  - x host-transposed to [128(feat), rows], quantized to fp8e4 (halves
    HBM traffic; weights stay bf16 - mixed-dtype matmul; row-quantization
    noise averages out over the 200k-row pooling, rel err ~1.4e-3).
  - L1: wct stationary, TILE=1280 rows split across two psum pools
    (ps_a 768 cols -> ACT relu, ps_d 512 cols -> DVE relu) so the two
    relu halves recycle their psum banks independently.
  - L4: h chunks (128 rows) as matmul stationary so pt lands
    rows-on-partitions; runs 2 tiles behind L1 so stationary loads are
    long-ready (they then pipeline at ~60 cycles/chunk).  pt accumulates
    in psum batches of 96 chunks and is DMAed straight to DRAM (f32).
"""

import numpy as np
import ml_dtypes

from concourse import bacc, mybir, tile
from concourse.bass_utils import run_bass_kernel_spmd

N_CORES = 8
N_ROWS = 200000
F = 128
OUT = 5

ROWS_PAD = 200704            # 8 * 25088
RPC = ROWS_PAD // N_CORES    # rows per core = 25088
T0 = 768                     # prologue tile (one ps_a tile)
TILE = 1280                  # steady tile: 768 (ps_a/ACT) + 512 (ps_d/DVE)
N_TILES = (RPC - T0) // TILE  # 19
A_COLS = 768                 # ACT relu cols per tile (6 chunks)
D_COLS = 512                 # DVE relu cols per tile (4 chunks)
GROUP = 3840                 # rows per steady-state DMA (3 tiles)
CHUNK = 128
N_CHUNKS = RPC // CHUNK      # 196
SLOTS = 96                   # pt chunks per psum batch
N_BATCH = (N_CHUNKS + SLOTS - 1) // SLOTS  # 3 (96+96+4)

F32 = mybir.dt.float32
BF16 = mybir.dt.bfloat16
FP8 = mybir.dt.float8e4
AF = mybir.ActivationFunctionType
ALU = mybir.AluOpType


def build_bass(has_bias=False):
    nc = bacc.Bacc()

    CONST_COLS = (F + OUT + 1) if has_bias else (F + OUT)
    cst_ext = nc.declare_dram_parameter(
        "cst", [F, CONST_COLS], BF16, isOutput=False
    )
    xt_ext = nc.declare_dram_parameter("xt", [F, RPC], FP8, isOutput=False)
    pt_ext = nc.declare_dram_parameter(
        "pt", [F, N_CHUNKS, OUT], F32, isOutput=True
    )

    with tile.TileContext(nc) as tc:
        with (
            tc.tile_pool(name="scratch", bufs=1) as scratch,
            tc.tile_pool(name="xin", bufs=3) as xin,
            tc.tile_pool(name="hbufl", bufs=3) as hbufl,
            tc.tile_pool(name="hbufr", bufs=3) as hbufr,
            tc.tile_pool(name="ptb", bufs=2) as ptb,
            tc.tile_pool(name="ps_a", bufs=2, space="PSUM") as ps_a,
            tc.tile_pool(name="ps_d", bufs=2, space="PSUM") as ps_d,
            tc.tile_pool(name="ps_p", bufs=2, space="PSUM") as ps_p,
        ):
            cstt = scratch.tile([F, CONST_COLS], BF16)
            nc.sync.dma_start(out=cstt[:], in_=cst_ext[:])
            wct = cstt[:, :F]
            w4at = cstt[:, F : F + OUT]
            nc.tensor.ldweights(wct)  # PE observes the const DMA early
            bc = None
            if has_bias:
                bc = scratch.tile([F, 1], F32)
                nc.vector.tensor_copy(bc[:], cstt[:, F + OUT : F + OUT + 1])

            x0 = scratch.tile([F, T0], FP8)
            nc.sync.dma_start(out=x0[:], in_=xt_ext[:, :T0])
            xg = []
            n_groups = (RPC - T0 + GROUP - 1) // GROUP
            for g in range(n_groups):
                c0 = T0 + g * GROUP
                cw = min(GROUP, RPC - c0)
                t = xin.tile([F, GROUP], FP8)
                nc.sync.dma_start(out=t[:, :cw], in_=xt_ext[:, c0 : c0 + cw])
                xg.append(t)

            state = {"chunk": 0, "pp": None}
            hbufs = []  # per tile: (hl, hr, a_ch, n_ch)

            def act_relu(dst, src):
                if has_bias:
                    nc.scalar.activation(dst, src, AF.Relu, bias=bc[:], scale=1.0)
                else:
                    nc.scalar.activation(dst, src, AF.Relu)

            def dve_relu(dst, src):
                if has_bias:
                    nc.vector.tensor_scalar(dst, src, bc[:], 0.0, ALU.add, ALU.max)
                else:
                    nc.vector.tensor_scalar_max(dst, src, 0.0)

            def do_l1_t0():
                ha = ps_a.tile([F, A_COLS], F32, tag="ha", name="ha")
                nc.tensor.matmul(ha[:, :512], wct, x0[:, :512], start=True, stop=True)
                nc.tensor.matmul(ha[:, 512:768], wct, x0[:, 512:768], start=True, stop=True)
                hl = hbufl.tile([F, A_COLS], BF16, tag="hl")
                hr = hbufr.tile([F, D_COLS], BF16, tag="hr")
                act_relu(hl[:, :384], ha[:, :384])
                dve_relu(hr[:, :384], ha[:, 384:768])
                hbufs.append((hl, hr, 3, 6))

            def do_l1(rhs):
                ha = ps_a.tile([F, A_COLS], F32, tag="ha", name="ha")
                hd = ps_d.tile([F, D_COLS], F32, tag="hd", name="hd")
                nc.tensor.matmul(ha[:, :512], wct, rhs[:, :512], start=True, stop=True)
                nc.tensor.matmul(ha[:, 512:768], wct, rhs[:, 512:768], start=True, stop=True)
                nc.tensor.matmul(hd[:], wct, rhs[:, 768:1280], start=True, stop=True)
                hl = hbufl.tile([F, A_COLS], BF16, tag="hl")
                hr = hbufr.tile([F, D_COLS], BF16, tag="hr")
                act_relu(hl[:], ha[:])
                dve_relu(hr[:], hd[:])
                hbufs.append((hl, hr, 6, 10))

            def do_l4(ti):
                hl, hr, a_ch, n_ch = hbufs[ti]
                for j in range(n_ch):
                    c = state["chunk"]
                    s = c % SLOTS
                    if s == 0:
                        state["pp"] = ps_p.tile(
                            [F, SLOTS, OUT], F32, tag="pp", name="pp"
                        )
                    if j < a_ch:
                        lhs = hl[:, j * CHUNK : (j + 1) * CHUNK]
                    else:
                        lhs = hr[:, (j - a_ch) * CHUNK : (j - a_ch + 1) * CHUNK]
                    nc.tensor.matmul(
                        state["pp"][:, s, :], lhs, w4at,
                        start=True, stop=True,
                    )
                    state["chunk"] = c + 1
                    if s == SLOTS - 1 or state["chunk"] == N_CHUNKS:
                        c0 = state["chunk"] - (s + 1)
                        bi = c0 // SLOTS
                        pts = ptb.tile([F, SLOTS, OUT], F32, tag="pts")
                        if bi % 2 == 0:
                            nc.vector.tensor_copy(
                                pts[:, : s + 1, :], state["pp"][:, : s + 1, :]
                            )
                        else:
                            nc.scalar.activation(
                                pts[:, : s + 1, :], state["pp"][:, : s + 1, :],
                                AF.Copy,
                            )
                        nc.sync.dma_start(
                            out=pt_ext[:, c0 : state["chunk"], :],
                            in_=pts[:, : s + 1, :],
                        )

            do_l1_t0()
            for t in range(N_TILES):
                g, off = divmod(t * TILE, GROUP)
                if off + TILE <= GROUP:
                    do_l1(xg[g][:, off : off + TILE])
                else:
                    # tile straddles two DMA groups (GROUP=3TILE so never)
                    raise AssertionError("tile crosses group boundary")
                if t >= 1:
                    do_l4(t - 1)
            do_l4(N_TILES - 1)
            do_l4(N_TILES)

    nc.finalize()
    return nc


_CACHED = {}
TRACE = False
LAST = {}


def kernel(x, W1, b1, W2, b2, W3, b3, W4, b4, alpha):
    f64 = np.float64
    x2 = np.asarray(x, np.float32).reshape(N_ROWS, F)
    W1, b1, W2, b2, W3, b3, W4, b4, alpha = [
        np.asarray(a, f64) for a in (W1, b1, W2, b2, W3, b3, W4, b4, alpha)
    ]

    # fold the linear layers (exact in f64)
    Wc = W3 @ W2 @ W1
    bc = W3 @ (W2 @ b1 + b2) + b3
    alpha_safe = np.where(np.abs(alpha) < 1e-12, 1e-12, alpha)
    W4a = alpha_safe[:, None] * W4

    # pad rows to 8*25088 with zeros; pad rows dropped after the gather
    n_pad = ROWS_PAD - N_ROWS
    xp = np.concatenate([x2, np.zeros((n_pad, F), np.float32)], axis=0)
    xT = np.ascontiguousarray(xp.T).astype(ml_dtypes.float8_e4m3fn)

    has_bias = bool(np.any(bc != 0.0))
    key = ("nc", has_bias)
    if key not in _CACHED:
        _CACHED[key] = build_bass(has_bias)
    nc = _CACHED[key]

    wct_np = np.ascontiguousarray(Wc.T).astype(ml_dtypes.bfloat16)
    w4at_np = np.ascontiguousarray(W4a.T).astype(ml_dtypes.bfloat16)
    parts_list = [wct_np, w4at_np]
    if has_bias:
        parts_list.append(
            bc.reshape(F, 1).astype(np.float32).astype(ml_dtypes.bfloat16)
        )
    consts_np = np.concatenate(parts_list, axis=1)

    in_maps = []
    for c in range(N_CORES):
        shard = np.ascontiguousarray(xT[:, c * RPC : (c + 1) * RPC])
        in_maps.append({"cst": consts_np, "xt": shard})

    res = run_bass_kernel_spmd(
        nc, in_maps, core_ids=list(range(N_CORES)), trace=TRACE
    )
    LAST["res"] = res

    # gather pt: per core [F(part=row-in-chunk), N_CHUNKS, OUT]
    pts = np.stack([np.asarray(r["pt"], np.float32) for r in res.results])
    # rows order: (core, chunk, partition)
    pt = pts.transpose(0, 2, 1, 3).reshape(ROWS_PAD, OUT).astype(f64)
    pt = pt[:N_ROWS]

    # host softmax pooling in f64:  out_o = sum pt*e^pt / (alpha*sum e^pt) + b4
    m = pt.max(axis=0)
    e = np.exp(pt - m)
    den = e.sum(axis=0)
    num = (pt * e).sum(axis=0)
    out = num / (alpha_safe * den) + b4
    return out[None, :].astype(np.float32)
